# revision 32
# baseline (speedup 1.0000x reference)
"""GraphSAGE (mean aggregation) on 8 Trainium2 NeuronCores.

Strategy (v6):
  - Nodes partitioned across 8 cores (6250 real + pad -> 6400/core).
  - Full node-feature table [51200, 128] fp16 replicated in each core's
    DRAM as two 25600-row half-tables; refreshed with two fp16 AllGathers
    per layer. The half-0 AllGather fires mid-layer (after window 24's
    writeback) so it overlaps the remaining windows' compute; token ids
    are remapped (half = local//3200) so each AllGather chunk lands
    exactly in one gather half-table.
  - Edge messages fetched with dma_gather (SWDGE, 4 queues, 1024-index
    chunks spanning window boundaries, 2-deep ring via 32K scratch,
    int16 indices). fp16 rows feed the PE directly (no f32->f16 copy).
  - Mean aggregation = PE matmuls: per 128-edge block, lhsT = gathered
    messages [128e, 128h] fp16, rhs = one-hot S [128e, 128]; accumulated
    in a [128h, 128-node] PSUM window. S built in one wide op per
    (half, window) via a 0-stride broadcast AP (DVE is_equal; every 8th
    group on ACT via |iota-dof| -> relu(1-t), exact) -- batching S into
    ~100 ops/layer instead of ~700 is the single biggest HW win.
  - In-degree reciprocals precomputed on host, resident in SBUF; mean =
    psum * inv (DVE tensor_tensor).
  - Dense SAGE transform per window: zT = Wl^T aggT + Wr^T hT; bias+relu
    on ACT; h' transposed back to node-major via PE and DMAed to cc_in.
"""
import sys

sys.path.insert(0, "/opt/trn_rl_repo")

import numpy as np

import concourse.bass as bass
import concourse.bacc as bacc
import concourse.tile as tile
from concourse import mybir, library_config
from concourse.masks import make_identity

# problem constants (hardcoded per contract)
N, E, IN_DIM, HID, L = 50000, 625000, 300, 128, 4
NC = 8
NPC = N // NC            # 6250 real nodes per core
W_N = 128                # aggregation window width (psum free dim)
NW = 50                  # windows per core
NPAD = W_N * NW          # 6400 padded nodes per core
NTAB = NC * NPAD         # 51200 rows in the replicated table
HALF = NTAB // 2         # 25600 (int16 index limit per gather table slice)
HLOC = NPAD // 2         # 3200 locals per AG chunk/table half
KCH = 3                  # 384 = 3*128 >= IN_DIM contraction chunks
GMAX = 1024              # max indices per dma_gather (descriptor carveout)
SCRATCH = 32768          # dynamic dma scratch (ring = SCRATCH/16 descs/queue)
ACT_MOD = 8              # blocks i with i % ACT_MOD >= ACT_CUT go to ACT
ACT_CUT = 8              # all S group builds on DVE (ACT: bias/relu only)
WB = 1                   # windows per dense/writeback batch


_CACHE = {}


def _host_prep(edge_index):
    """Build per-core gather streams, dst-offset blocks and program structure."""
    src = edge_index[0].astype(np.int64)
    dst = edge_index[1].astype(np.int64)
    # padded global ids
    gsrc = (src // NPC) * NPAD + (src % NPC)
    gdst = (dst // NPC) * NPAD + (dst % NPC)

    # token id within its half-table: half h = local//HLOC holds
    # rows m*HLOC + local%HLOC  (AG chunk h writes exactly half-table h)
    src_m = gsrc // NPAD
    src_loc = gsrc % NPAD
    src_half = src_loc // HLOC
    src_tok = src_m * HLOC + (src_loc % HLOC)

    per_core = []
    counts = np.zeros((NC, 2, NW), dtype=np.int64)
    for m in range(NC):
        sel = (gdst // NPAD) == m
        s_tok = src_tok[sel]
        dl = (gdst[sel] - m * NPAD).astype(np.int64)   # 0..6249
        half = src_half[sel]
        w = dl // W_N
        order = np.lexsort((dl, half))
        s_tok, dl, half, w = s_tok[order], dl[order], half[order], w[order]
        per_core.append((s_tok, dl, half, w))
        for h in range(2):
            cw = np.bincount(w[half == h], minlength=NW)
            counts[m, h, :] = cw

    # uniform block structure across cores
    B = np.zeros((2, NW), dtype=np.int64)
    for h in range(2):
        for w in range(NW):
            B[h, w] = int(np.ceil(counts[:, h, w].max() / 128.0))

    slots_h = [int(B[h].sum() * 128) for h in range(2)]
    nb_h = [int(B[h].sum()) for h in range(2)]

    # per-(h,w) slot offsets into the half stream
    slot_off = np.zeros((2, NW), dtype=np.int64)
    for h in range(2):
        acc = 0
        for w in range(NW):
            slot_off[h, w] = acc
            acc += B[h, w] * 128

    # gather chunks per half: runs of <= GMAX slots spanning window bounds
    chunks = [[], []]           # per half: list of (slot_off, n)
    for h in range(2):
        off, left = 0, slots_h[h]
        while left > 0:
            n = min(GMAX, left)
            chunks[h].append((off, n))
            off += n
            left -= n

    # map: (h, global_block) -> (chunk_id, j within chunk)
    blk_map = [[], []]
    for h in range(2):
        for b in range(nb_h[h]):
            s = b * 128
            for ci, (w0, n) in enumerate(chunks[h]):
                if w0 <= s < w0 + n:
                    blk_map[h].append((ci, (s - w0) // 128))
                    break

    # in-degrees (padded local layout) for the mean
    deg_all = np.bincount(gdst, minlength=NTAB).astype(np.float32)
    inv_all = 1.0 / np.maximum(deg_all, 1.0)

    idx_wrapped = []   # per core: [2][128, slots_h/16] int16
    dof_arr = []       # per core: [2][128, nb_h] fp32
    for m in range(NC):
        s_m, dl, half, w = per_core[m]
        iw_pair, dof_pair = [], []
        for h in range(2):
            tok = np.zeros(slots_h[h], dtype=np.int16)   # pad -> row 0
            dof = np.full(slots_h[h], -1.0, dtype=np.float32)
            sel = half == h
            s_h, dl_h, w_h = s_m[sel], dl[sel], w[sel]
            for wi in range(NW):
                selw = w_h == wi
                cnt = int(selw.sum())
                if cnt == 0:
                    continue
                o = int(slot_off[h, wi])
                tok[o : o + cnt] = s_h[selw].astype(np.int16)
                dof[o : o + cnt] = (dl_h[selw] - wi * W_N).astype(np.float32)
            # wrap idx per gather instruction: j -> [j%16, j//16], replicated x8
            iw = np.zeros((128, max(slots_h[h] // 16, 1)), dtype=np.int16)
            for w0, n in chunks[h]:
                blockw = tok[w0 : w0 + n].reshape(n // 16, 16).T  # [16, n/16]
                iw[:, w0 // 16 : (w0 + n) // 16] = np.tile(blockw, (8, 1))
            iw_pair.append(iw)
            # dstoff partition-major: dof_arr[p, b] = dof[b*128+p]
            dof_pair.append(
                np.ascontiguousarray(dof.reshape(max(nb_h[h], 1), 128).T)
            )
        idx_wrapped.append(iw_pair)
        dof_arr.append(dof_pair)

    return {
        "B": B,
        "slots_h": slots_h,
        "nb_h": nb_h,
        "slot_off": slot_off,
        "chunks": chunks,
        "blk_map": blk_map,
        "idx_wrapped": idx_wrapped,
        "dof": dof_arr,
        "inv_all": inv_all,
    }


def _build_program(struct, timing_reps=1):
    B = struct["B"]
    slots_h = struct["slots_h"]
    nb_h = struct["nb_h"]
    chunks = struct["chunks"]
    blk_map = struct["blk_map"]

    nc = bacc.Bacc(
        "TRN2",
        target_bir_lowering=False,
        debug=False,
        num_devices=NC,
        num_swdge_queues=4,
        dynamic_dma_scratch_size=SCRATCH,
    )
    f32, f16, i16 = mybir.dt.float32, mybir.dt.float16, mybir.dt.int16

    idx_d = [
        nc.dram_tensor(f"idx{h}", [128, max(slots_h[h] // 16, 1)], i16, kind="ExternalInput")
        for h in range(2)
    ]
    dof_d = [
        nc.dram_tensor(f"dof{h}", [128, max(nb_h[h], 1)], f32, kind="ExternalInput")
        for h in range(2)
    ]
    xT_d = nc.dram_tensor("xT", [KCH, 128, NPAD], f16, kind="ExternalInput")
    embW_d = nc.dram_tensor("embW", [KCH, 128, HID], f16, kind="ExternalInput")
    embB_d = nc.dram_tensor("embB", [128, 1], f32, kind="ExternalInput")
    Wl_d = nc.dram_tensor("Wl", [L, 128, HID], f16, kind="ExternalInput")
    Wr_d = nc.dram_tensor("Wr", [L, 128, HID], f16, kind="ExternalInput")
    bl_d = nc.dram_tensor("bl", [L, 128, 1], f32, kind="ExternalInput")
    iota_d = nc.dram_tensor("iota", [128, 8 * W_N], f16, kind="ExternalInput")
    inv_d = nc.dram_tensor("inv", [128, NPAD], f32, kind="ExternalInput")
    out_d = nc.dram_tensor("out", [NPAD, HID], f32, kind="ExternalOutput")

    rg = [list(range(NC))]
    qctr = [0]

    def next_q():
        q = qctr[0] % 4
        qctr[0] += 1
        return q

    import os as _os
    _trace = _os.environ.get("KERNEL_TRACE_SIM") == "1"
    _ablate = _os.environ.get("KERNEL_ABLATE") == "1"
    _no_ag = _os.environ.get("KERNEL_NO_AG") == "1"
    _sbatch = _os.environ.get("KERNEL_SBATCH", "1") == "1"
    _wbwide = _os.environ.get("KERNEL_WBWIDE", "1") == "1"
    with tile.TileContext(nc, trace_sim=_trace) as tc:
        with (
            tc.tile_pool(name="const", bufs=1) as constp,
            tc.tile_pool(name="big", bufs=1) as bigp,
            tc.tile_pool(name="xw", bufs=3) as xwp,
            tc.tile_pool(name="gt16", bufs=6) as gt16p,
            tc.tile_pool(name="sp", bufs=8) as sp,
            tc.tile_pool(name="tmp", bufs=4) as tmpp,
            tc.tile_pool(name="ap", bufs=4) as apool,
            tc.tile_pool(name="hp", bufs=4) as hpool,
            tc.tile_pool(name="pag", bufs=5, space="PSUM") as pag,
            tc.tile_pool(name="pz", bufs=1, space="PSUM") as pz,
            tc.tile_pool(name="pt", bufs=2, space="PSUM") as pt,
            tc.tile_pool(name="dram", bufs=1, space="DRAM") as dram,
        ):
            nc.gpsimd.load_library(library_config.mlp)

            # --- resident constants / inputs in SBUF ---
            idx_sb = []
            dof_sb = []
            for h in range(2):
                t = constp.tile([128, max(slots_h[h] // 16, 1)], i16, name=f"idxsb{h}")
                nc.sync.dma_start(out=t[:], in_=idx_d[h][:])
                idx_sb.append(t)
                t2 = constp.tile([128, max(nb_h[h], 1)], f32, name=f"dofsb{h}")
                nc.sync.dma_start(out=t2[:], in_=dof_d[h][:])
                dof_sb.append(t2)
            iota_sb = constp.tile([128, 8 * W_N], f16)
            nc.sync.dma_start(out=iota_sb[:], in_=iota_d[:])
            ident = constp.tile([128, 128], f32)
            make_identity(nc, ident[:])
            ident16 = constp.tile([128, 128], f16)
            nc.vector.tensor_copy(ident16[:], ident[:])
            embW_sb = constp.tile([128, KCH, HID], f16)
            nc.sync.dma_start(out=embW_sb[:], in_=embW_d[:].rearrange("k p h -> p k h"))
            embB_sb = constp.tile([128, 1], f32)
            nc.sync.dma_start(out=embB_sb[:], in_=embB_d[:])
            Wl_sb = constp.tile([128, L, HID], f16)
            nc.sync.dma_start(out=Wl_sb[:], in_=Wl_d[:].rearrange("l p h -> p l h"))
            Wr_sb = constp.tile([128, L, HID], f16)
            nc.sync.dma_start(out=Wr_sb[:], in_=Wr_d[:].rearrange("l p h -> p l h"))
            bl_sb = constp.tile([128, L], f32)
            nc.sync.dma_start(out=bl_sb[:], in_=bl_d[:].rearrange("l p one -> p (l one)"))
            inv_sb = bigp.tile([128, NPAD], f32)
            nc.sync.dma_start(out=inv_sb[:], in_=inv_d[:])

            hT = [bigp.tile([128, NPAD], f16, name=f"hT{i}") for i in range(2)]

            # DRAM buffers: per AG round, one tile per half-table chunk
            # (Shared-space DRAM requires a single writer per tile)
            n_ag = timing_reps * L
            cc_in = [
                dram.tile([NPAD, HID], f16, name=f"ccin{i}", bufs=1) for i in range(2)
            ]
            h_half = [
                [
                    dram.tile(
                        [HALF, HID], f16, name=f"hall{i}h{h}", bufs=1,
                        addr_space="Shared",
                    )
                    for h in range(2)
                ]
                for i in range(n_ag)
            ]

            def build_S_group(h, boff, nb, use_act):
                # one-hot S for nb consecutive blocks in one wide op:
                # S[p, b*W_N + c] = (iota[c] == dof[p, boff+b])
                S = sp.tile([128, nb * W_N], f16, tag="S", name="S")
                in0 = iota_sb[:, 0 : nb * W_N].rearrange(
                    "p (b c) -> p b c", c=W_N
                )
                d = dof_sb[h][:, boff : boff + nb]
                in1 = bass.AP(d.tensor, d.offset, list(d.ap) + [[0, W_N]])
                out3 = S[:].rearrange("p (b c) -> p b c", c=W_N)
                if use_act:
                    t = tmpp.tile([128, nb * W_N], f16, tag="T", name="T")
                    t3 = t[:].rearrange("p (b c) -> p b c", c=W_N)
                    # t = iota - dof ; S = relu(1 - |t|)  (exact one-hot)
                    nc.vector.tensor_tensor(
                        out=t3, in0=in0, in1=in1,
                        op=mybir.AluOpType.subtract,
                    )
                    nc.scalar.activation(
                        S[:], t[:], mybir.ActivationFunctionType.Abs,
                    )
                    nc.scalar.activation(
                        S[:], S[:], mybir.ActivationFunctionType.Relu,
                        bias=1.0, scale=-1.0,
                    )
                else:
                    nc.vector.tensor_tensor(
                        out=out3, in0=in0, in1=in1,
                        op=mybir.AluOpType.is_equal,
                    )
                return S

            def build_S(h, b_global, use_act):
                S = sp.tile([128, W_N], f16, tag="S", name="S")
                if use_act:
                    t = tmpp.tile([128, W_N], f16, tag="T", name="T")
                    # t = |dof - iota| ; S = relu(1 - t)  (exact one-hot)
                    nc.scalar.activation(
                        t[:], iota_sb[:, 0:W_N], mybir.ActivationFunctionType.Abs,
                        bias=dof_sb[h][:, b_global : b_global + 1], scale=-1.0,
                    )
                    nc.scalar.activation(
                        S[:], t[:], mybir.ActivationFunctionType.Relu,
                        bias=1.0, scale=-1.0,
                    )
                else:
                    nc.vector.tensor_scalar(
                        out=S[:],
                        in0=iota_sb[:, 0:W_N],
                        scalar1=dof_sb[h][:, b_global : b_global + 1],
                        scalar2=None,
                        op0=mybir.AluOpType.is_equal,
                    )
                return S

            def writeback_wide(hTbuf, w0, dest, last=False):
                cs = slice(w0 * W_N, (w0 + WB) * W_N)
                if last:
                    hsb = hpool.tile([128, WB * 128], f32, tag="hsbw32", name="hsbw32")
                else:
                    hsb = hpool.tile([128, WB * 128], f16, tag="hsbw", name="hsbw")
                for q in range(WB):
                    ptile = pt.tile([128, 128], f16, tag="pt16", name="ptile")
                    nc.tensor.transpose(
                        ptile[:],
                        hTbuf[:, (w0 + q) * W_N : (w0 + q + 1) * W_N],
                        ident16[:],
                    )
                    nc.any.tensor_copy(hsb[:, q * 128 : (q + 1) * 128], ptile[:])
                nc.sync.dma_start(
                    out=dest[cs, :].rearrange("(q p) n -> p q n", q=WB),
                    in_=hsb[:].rearrange("p (q n) -> p q n", q=WB),
                )

            def writeback(hTbuf, w, dest, last=False):
                # transpose window back to node-major and DMA to dest rows
                cs = slice(w * W_N, (w + 1) * W_N)
                ptile = pt.tile([128, 128], f16, tag="pt16", name="ptile")
                nc.tensor.transpose(ptile[:], hTbuf[:, cs], ident16[:])
                if last:
                    hsb = hpool.tile([128, 128], f32, tag="hsb32", name="hsb32")
                else:
                    hsb = hpool.tile([128, 128], f16, tag="hsb", name="hsb")
                nc.any.tensor_copy(hsb[:], ptile[:])
                nc.sync.dma_start(out=dest[cs, :], in_=hsb[:])

            def embedding(ag_pair):
                for w in range(NW):
                    ws = slice(w * W_N, (w + 1) * W_N)
                    xw = xwp.tile([128, KCH, W_N], f16, tag="xw", name="xw")
                    nc.sync.dma_start(
                        out=xw[:], in_=xT_d[:, :, ws].rearrange("k p n -> p k n")
                    )
                    pzz = pz.tile([128, W_N], f32, tag="pz", name="pz")
                    for k in range(KCH):
                        nc.tensor.matmul(
                            pzz[:],
                            lhsT=embW_sb[:, k, :],
                            rhs=xw[:, k, :],
                            start=(k == 0),
                            stop=(k == KCH - 1),
                        )
                    nc.scalar.activation(
                        hT[0][:, ws], pzz[:], mybir.ActivationFunctionType.Relu,
                        bias=embB_sb[:],
                    )
                    writeback(hT[0], w, cc_in[0])
                    if w in (NW // 2 - 1, NW - 1):
                        h = 0 if w == NW // 2 - 1 else 1
                        allgather_half(ag_pair[0], ag_pair[1], h)

            def layer(l, h_src, hT_in, hT_out, dest, last, ag_pair=None):
                half_ap = [h_src[0][:], h_src[1][:]]
                # issue gather chunks lazily as windows consume them
                gts = [[None] * len(chunks[0]), [None] * len(chunks[1])]

                def ensure_chunk(h, ci):
                    if gts[h][ci] is None:
                        w0, n = chunks[h][ci]
                        gt = gt16p.tile([128, n // 128, 128], f16, tag="gt", name="gt")
                        nc.gpsimd.dma_gather(
                            gt[:],
                            half_ap[h],
                            idx_sb[h][:, w0 // 16 : (w0 + n) // 16],
                            n,
                            n,
                            HID,
                            queue_num=next_q(),
                        )
                        gts[h][ci] = gt
                    return gts[h][ci]

                blk_ctr = 0
                for wg in range(NW // WB):
                    w0 = wg * WB
                    ws = slice(w0 * W_N, (w0 + WB) * W_N)
                    aggT = apool.tile([128, WB * W_N], f16, tag="aggT", name="aggT")
                    for wi in range(WB):
                        w = w0 + wi
                        pagg = pag.tile([128, W_N], f32, tag="pagg", name="pagg")
                        nblocks = int(B[0][w] + B[1][w])
                        done = 0
                        first = True
                        for h in range(2):
                            boff = int(B[h][:w].sum())
                            nb = int(B[h][w])
                            if _ablate:
                                for lb in range(nb):
                                    ci, j = blk_map[h][boff + lb]
                                    ensure_chunk(h, ci)
                                done += nb
                                continue
                            if nb == 0:
                                continue
                            use_act = (blk_ctr % ACT_MOD) >= ACT_CUT
                            blk_ctr += 1
                            Sg = build_S_group(h, boff, nb, use_act)
                            for lb in range(nb):
                                ci, j = blk_map[h][boff + lb]
                                gt = ensure_chunk(h, ci)
                                done += 1
                                nc.tensor.matmul(
                                    pagg[:],
                                    lhsT=gt[:, j, :],
                                    rhs=Sg[:, lb * W_N : (lb + 1) * W_N],
                                    start=first,
                                    stop=(done == nblocks),
                                )
                                first = False
                        asl = aggT[:, wi * W_N : (wi + 1) * W_N]
                        if first:
                            nc.vector.memset(asl, 0.0)
                        else:
                            nc.vector.tensor_tensor(
                                out=asl,
                                in0=pagg[:],
                                in1=inv_sb[:, (w0 + wi) * W_N : (w0 + wi + 1) * W_N],
                                op=mybir.AluOpType.mult,
                            )
                    pzz = pz.tile([128, WB * W_N], f32, tag="pz", name="pz")
                    nc.tensor.matmul(
                        pzz[:], lhsT=Wl_sb[:, l, :], rhs=aggT[:], start=True, stop=False
                    )
                    nc.tensor.matmul(
                        pzz[:], lhsT=Wr_sb[:, l, :], rhs=hT_in[:, ws], start=False,
                        stop=True,
                    )
                    nc.scalar.activation(
                        hT_out[:, ws], pzz[:], mybir.ActivationFunctionType.Relu,
                        bias=bl_sb[:, l : l + 1],
                    )
                    for q in range(WB):
                        writeback(hT_out, w0 + q, dest, last=last)
                    if ag_pair is not None and wg in (
                        (NW // 2 - 1) // WB, NW // WB - 1
                    ):
                        h = 0 if wg == (NW // 2 - 1) // WB else 1
                        allgather_half(ag_pair[0], ag_pair[1], h)

            def allgather_half(src_cc, dst_pair, h):
                if _no_ag:
                    return
                rs = slice(h * (NPAD // 2), (h + 1) * (NPAD // 2))
                nc.gpsimd.collective_compute(
                    "AllGather",
                    mybir.AluOpType.bypass,
                    replica_groups=rg,
                    ins=[src_cc[rs, :]],
                    outs=[dst_pair[h][:]],
                )

            embedding(ag_pair=(cc_in[0], h_half[0]))
            agi = 0
            nlay = timing_reps * L
            for li in range(nlay):
                l = li % L
                last = li == nlay - 1
                layer(
                    l,
                    h_half[agi],
                    hT[li % 2],
                    hT[(li + 1) % 2],
                    out_d if last else cc_in[(li + 1) % 2],
                    last,
                    ag_pair=None if last else (cc_in[(li + 1) % 2], h_half[agi + 1]),
                )
                if not last:
                    agi += 1

    nc.compile()
    return nc


def _prep_inputs(inputs, struct):
    x = np.asarray(inputs["x"], dtype=np.float32)
    emb_W = np.asarray(inputs["emb_W"], dtype=np.float32)
    emb_b = np.asarray(inputs["emb_b"], dtype=np.float32)
    Wl = np.asarray(inputs["Wl"], dtype=np.float32)
    bl = np.asarray(inputs["bl"], dtype=np.float32)
    Wr = np.asarray(inputs["Wr"], dtype=np.float32)

    embW_p = np.zeros((KCH, 128, HID), dtype=np.float16)
    embW_p.reshape(KCH * 128, HID)[:IN_DIM] = emb_W.astype(np.float16)
    embB_p = np.zeros((128, 1), dtype=np.float32)
    embB_p[:, 0] = emb_b
    Wl_p = Wl.astype(np.float16)
    Wr_p = Wr.astype(np.float16)
    bl_p = np.ascontiguousarray(bl[:, :, None].astype(np.float32))

    iota = np.broadcast_to(
        np.tile(np.arange(W_N, dtype=np.float16), 8)[None, :], (128, 8 * W_N)
    ).copy()

    inv_all = struct["inv_all"]

    in_maps = []
    for m in range(NC):
        xm = np.zeros((KCH * 128, NPAD), dtype=np.float16)
        xm[:IN_DIM, :NPC] = x[m * NPC : (m + 1) * NPC].T.astype(np.float16)
        inv_m = np.broadcast_to(
            inv_all[m * NPAD : (m + 1) * NPAD][None, :], (128, NPAD)
        ).copy()
        im = {
            "idx0": struct["idx_wrapped"][m][0],
            "idx1": struct["idx_wrapped"][m][1],
            "dof0": struct["dof"][m][0],
            "dof1": struct["dof"][m][1],
            "xT": xm.reshape(KCH, 128, NPAD),
            "embW": embW_p,
            "embB": embB_p,
            "Wl": Wl_p,
            "Wr": Wr_p,
            "bl": bl_p,
            "iota": iota,
            "inv": inv_m,
        }
        in_maps.append(im)
    return in_maps


class BassRunner:
    """Executes a compiled Bass program via PJRT/axon; jit built once."""

    def __init__(self, nc, n_cores):
        import jax
        from jax.sharding import Mesh, PartitionSpec
        from jax.experimental.shard_map import shard_map
        from concourse.bass2jax import (
            _bass_exec_p,
            install_neuronx_cc_hook,
            partition_id_tensor,
        )

        install_neuronx_cc_hook()
        self.jax = jax
        self.nc = nc
        self.n_cores = n_cores
        partition_name = (
            nc.partition_id_tensor.name if nc.partition_id_tensor else None
        )
        in_names, out_names, out_avals, zero_outs = [], [], [], []
        for alloc in nc.m.functions[0].allocations:
            if not isinstance(alloc, mybir.MemoryLocationSet):
                continue
            name = alloc.memorylocations[0].name
            if alloc.kind == "ExternalInput":
                if name != partition_name:
                    in_names.append(name)
            elif alloc.kind == "ExternalOutput":
                shape = tuple(alloc.tensor_shape)
                dtype = mybir.dt.np(alloc.dtype)
                out_names.append(name)
                out_avals.append(jax.core.ShapedArray(shape, dtype))
                zero_outs.append(np.zeros(shape, dtype))
        self.in_names, self.out_names = in_names, out_names
        self.zero_outs, self._out_avals = zero_outs, out_avals
        n_params, n_outs = len(in_names), len(out_avals)
        all_in_names = in_names + out_names
        if partition_name is not None:
            all_in_names = all_in_names + [partition_name]

        def _body(*args):
            operands = list(args)
            if partition_name is not None:
                operands.append(partition_id_tensor())
            return tuple(
                _bass_exec_p.bind(
                    *operands,
                    out_avals=tuple(out_avals),
                    in_names=tuple(all_in_names),
                    out_names=tuple(out_names),
                    lowering_input_output_aliases=(),
                    sim_require_finite=True,
                    sim_require_nnan=True,
                    nc=nc,
                )
            )

        devices = jax.devices()[:n_cores]
        self._mesh = Mesh(np.asarray(devices), ("core",))
        self._pspec = PartitionSpec("core")
        in_specs = (self._pspec,) * (n_params + n_outs)
        out_specs = (self._pspec,) * len(out_names)
        self._fn = jax.jit(
            shard_map(
                _body,
                mesh=self._mesh,
                in_specs=in_specs,
                out_specs=out_specs,
                check_rep=False,
            ),
            keep_unused=True,
        )

    def prepare(self, in_maps):
        n = self.n_cores
        concat_in = [
            np.concatenate(
                [np.asarray(in_maps[c][name]) for c in range(n)], axis=0
            )
            for name in self.in_names
        ]
        concat_zeros = [
            np.zeros((n * z.shape[0], *z.shape[1:]), z.dtype)
            for z in self.zero_outs
        ]
        sharding = self.jax.sharding.NamedSharding(self._mesh, self._pspec)
        self._args = [
            self.jax.device_put(a, sharding) for a in concat_in + concat_zeros
        ]

    def execute(self):
        outs = self._fn(*self._args)
        self.jax.block_until_ready(outs)
        return outs

    def run(self):
        outs = self.execute()
        n = self.n_cores
        return [
            {
                name: np.asarray(outs[i]).reshape(
                    n, *self._out_avals[i].shape
                )[c]
                for i, name in enumerate(self.out_names)
            }
            for c in range(n)
        ]


def _get_runner(edge_index, timing_reps=1):
    import os
    key = ("prog", timing_reps, os.environ.get("KERNEL_NO_AG"),
           os.environ.get("KERNEL_ABLATE"), hash(edge_index.tobytes()))
    if key in _CACHE:
        return _CACHE[key]
    struct = _host_prep(edge_index)
    nc = _build_program(struct, timing_reps=timing_reps)
    runner = BassRunner(nc, NC)
    _CACHE[key] = (struct, runner)
    return struct, runner


def kernel(**inputs):
    edge_index = np.asarray(inputs["edge_index"])
    struct, runner = _get_runner(edge_index)
    in_maps = _prep_inputs(inputs, struct)
    runner.prepare(in_maps)
    results = runner.run()
    out = np.empty((N, HID), dtype=np.float32)
    for m in range(NC):
        out[m * NPC : (m + 1) * NPC] = results[m]["out"][:NPC]
    return out


# revision 33
# speedup vs baseline: 1.0922x; 1.0922x over previous
"""GraphSAGE (mean aggregation) on 8 Trainium2 NeuronCores.

Strategy (v6):
  - Nodes partitioned across 8 cores (6250 real + pad -> 6400/core).
  - Full node-feature table [51200, 128] fp16 replicated in each core's
    DRAM as two 25600-row half-tables; refreshed with two fp16 AllGathers
    per layer. The half-0 AllGather fires mid-layer (after window 24's
    writeback) so it overlaps the remaining windows' compute; token ids
    are remapped (half = local//3200) so each AllGather chunk lands
    exactly in one gather half-table.
  - Edge messages fetched with dma_gather (SWDGE, 4 queues, 1024-index
    chunks spanning window boundaries, 2-deep ring via 32K scratch,
    int16 indices). fp16 rows feed the PE directly (no f32->f16 copy).
  - Mean aggregation = PE matmuls: per 128-edge block, lhsT = gathered
    messages [128e, 128h] fp16, rhs = one-hot S [128e, 128]; accumulated
    in a [128h, 128-node] PSUM window. S built in one wide op per
    (half, window) via a 0-stride broadcast AP (DVE is_equal; every 8th
    group on ACT via |iota-dof| -> relu(1-t), exact) -- batching S into
    ~100 ops/layer instead of ~700 is the single biggest HW win.
  - In-degree reciprocals precomputed on host, resident in SBUF; mean =
    psum * inv (DVE tensor_tensor).
  - Dense SAGE transform per window: zT = Wl^T aggT + Wr^T hT; bias+relu
    on ACT; h' transposed back to node-major via PE and DMAed to cc_in.
"""
import sys

sys.path.insert(0, "/opt/trn_rl_repo")

import numpy as np

import concourse.bass as bass
import concourse.bacc as bacc
import concourse.tile as tile
from concourse import mybir, library_config
from concourse.masks import make_identity

# problem constants (hardcoded per contract)
N, E, IN_DIM, HID, L = 50000, 625000, 300, 128, 4
NC = 8
NPC = N // NC            # 6250 real nodes per core
W_N = 128                # aggregation window width (psum free dim)
NW = 50                  # windows per core
NPAD = W_N * NW          # 6400 padded nodes per core
NTAB = NC * NPAD         # 51200 rows in the replicated table
HALF = NTAB // 2         # 25600 (int16 index limit per gather table slice)
HLOC = NPAD // 2         # 3200 locals per AG chunk/table half
KCH = 3                  # 384 = 3*128 >= IN_DIM contraction chunks
GMAX = 1024              # max indices per dma_gather (descriptor carveout)
SCRATCH = 32768          # dynamic dma scratch (ring = SCRATCH/16 descs/queue)
ACT_MOD = 8              # blocks i with i % ACT_MOD >= ACT_CUT go to ACT
ACT_CUT = 7              # 7/8 of S group builds on DVE, 1/8 on ACT
WB = 1                   # windows per dense/writeback batch


_CACHE = {}


def _host_prep(edge_index):
    """Build per-core gather streams, dst-offset blocks and program structure."""
    src = edge_index[0].astype(np.int64)
    dst = edge_index[1].astype(np.int64)
    # padded global ids
    gsrc = (src // NPC) * NPAD + (src % NPC)
    gdst = (dst // NPC) * NPAD + (dst % NPC)

    # token id within its half-table: half h = local//HLOC holds
    # rows m*HLOC + local%HLOC  (AG chunk h writes exactly half-table h)
    src_m = gsrc // NPAD
    src_loc = gsrc % NPAD
    src_half = src_loc // HLOC
    src_tok = src_m * HLOC + (src_loc % HLOC)

    per_core = []
    counts = np.zeros((NC, 2, NW), dtype=np.int64)
    for m in range(NC):
        sel = (gdst // NPAD) == m
        s_tok = src_tok[sel]
        dl = (gdst[sel] - m * NPAD).astype(np.int64)   # 0..6249
        half = src_half[sel]
        w = dl // W_N
        order = np.lexsort((dl, half))
        s_tok, dl, half, w = s_tok[order], dl[order], half[order], w[order]
        per_core.append((s_tok, dl, half, w))
        for h in range(2):
            cw = np.bincount(w[half == h], minlength=NW)
            counts[m, h, :] = cw

    # uniform block structure across cores
    B = np.zeros((2, NW), dtype=np.int64)
    for h in range(2):
        for w in range(NW):
            B[h, w] = int(np.ceil(counts[:, h, w].max() / 128.0))

    slots_h = [int(B[h].sum() * 128) for h in range(2)]
    nb_h = [int(B[h].sum()) for h in range(2)]

    # per-(h,w) slot offsets into the half stream
    slot_off = np.zeros((2, NW), dtype=np.int64)
    for h in range(2):
        acc = 0
        for w in range(NW):
            slot_off[h, w] = acc
            acc += B[h, w] * 128

    # gather chunks per half: runs of <= GMAX slots spanning window bounds
    chunks = [[], []]           # per half: list of (slot_off, n)
    for h in range(2):
        off, left = 0, slots_h[h]
        while left > 0:
            n = min(GMAX, left)
            chunks[h].append((off, n))
            off += n
            left -= n

    # map: (h, global_block) -> (chunk_id, j within chunk)
    blk_map = [[], []]
    for h in range(2):
        for b in range(nb_h[h]):
            s = b * 128
            for ci, (w0, n) in enumerate(chunks[h]):
                if w0 <= s < w0 + n:
                    blk_map[h].append((ci, (s - w0) // 128))
                    break

    # in-degrees (padded local layout) for the mean
    deg_all = np.bincount(gdst, minlength=NTAB).astype(np.float32)
    inv_all = 1.0 / np.maximum(deg_all, 1.0)

    idx_wrapped = []   # per core: [2][128, slots_h/16] int16
    dof_arr = []       # per core: [2][128, nb_h] fp32
    for m in range(NC):
        s_m, dl, half, w = per_core[m]
        iw_pair, dof_pair = [], []
        for h in range(2):
            tok = np.zeros(slots_h[h], dtype=np.int16)   # pad -> row 0
            dof = np.full(slots_h[h], -1.0, dtype=np.float32)
            sel = half == h
            s_h, dl_h, w_h = s_m[sel], dl[sel], w[sel]
            for wi in range(NW):
                selw = w_h == wi
                cnt = int(selw.sum())
                if cnt == 0:
                    continue
                o = int(slot_off[h, wi])
                tok[o : o + cnt] = s_h[selw].astype(np.int16)
                dof[o : o + cnt] = (dl_h[selw] - wi * W_N).astype(np.float32)
            # wrap idx per gather instruction: j -> [j%16, j//16], replicated x8
            iw = np.zeros((128, max(slots_h[h] // 16, 1)), dtype=np.int16)
            for w0, n in chunks[h]:
                blockw = tok[w0 : w0 + n].reshape(n // 16, 16).T  # [16, n/16]
                iw[:, w0 // 16 : (w0 + n) // 16] = np.tile(blockw, (8, 1))
            iw_pair.append(iw)
            # dstoff partition-major: dof_arr[p, b] = dof[b*128+p]
            dof_pair.append(
                np.ascontiguousarray(dof.reshape(max(nb_h[h], 1), 128).T)
            )
        idx_wrapped.append(iw_pair)
        dof_arr.append(dof_pair)

    return {
        "B": B,
        "slots_h": slots_h,
        "nb_h": nb_h,
        "slot_off": slot_off,
        "chunks": chunks,
        "blk_map": blk_map,
        "idx_wrapped": idx_wrapped,
        "dof": dof_arr,
        "inv_all": inv_all,
    }


def _build_program(struct, timing_reps=1):
    B = struct["B"]
    slots_h = struct["slots_h"]
    nb_h = struct["nb_h"]
    chunks = struct["chunks"]
    blk_map = struct["blk_map"]

    nc = bacc.Bacc(
        "TRN2",
        target_bir_lowering=False,
        debug=False,
        num_devices=NC,
        num_swdge_queues=4,
        dynamic_dma_scratch_size=SCRATCH,
    )
    f32, f16, i16 = mybir.dt.float32, mybir.dt.float16, mybir.dt.int16

    idx_d = [
        nc.dram_tensor(f"idx{h}", [128, max(slots_h[h] // 16, 1)], i16, kind="ExternalInput")
        for h in range(2)
    ]
    dof_d = [
        nc.dram_tensor(f"dof{h}", [128, max(nb_h[h], 1)], f32, kind="ExternalInput")
        for h in range(2)
    ]
    xT_d = nc.dram_tensor("xT", [KCH, 128, NPAD], f16, kind="ExternalInput")
    embW_d = nc.dram_tensor("embW", [KCH, 128, HID], f16, kind="ExternalInput")
    embB_d = nc.dram_tensor("embB", [128, 1], f32, kind="ExternalInput")
    Wl_d = nc.dram_tensor("Wl", [L, 128, HID], f16, kind="ExternalInput")
    Wr_d = nc.dram_tensor("Wr", [L, 128, HID], f16, kind="ExternalInput")
    bl_d = nc.dram_tensor("bl", [L, 128, 1], f32, kind="ExternalInput")
    iota_d = nc.dram_tensor("iota", [128, 8 * W_N], f16, kind="ExternalInput")
    inv_d = nc.dram_tensor("inv", [128, NPAD], f32, kind="ExternalInput")
    out_d = nc.dram_tensor("out", [NPAD, HID], f32, kind="ExternalOutput")

    rg = [list(range(NC))]
    qctr = [0]

    def next_q():
        q = qctr[0] % 4
        qctr[0] += 1
        return q

    import os as _os
    _trace = _os.environ.get("KERNEL_TRACE_SIM") == "1"
    _ablate = _os.environ.get("KERNEL_ABLATE") == "1"
    _no_ag = _os.environ.get("KERNEL_NO_AG") == "1"
    _sbatch = _os.environ.get("KERNEL_SBATCH", "1") == "1"
    _wbwide = _os.environ.get("KERNEL_WBWIDE", "1") == "1"
    with tile.TileContext(nc, trace_sim=_trace) as tc:
        with (
            tc.tile_pool(name="const", bufs=1) as constp,
            tc.tile_pool(name="big", bufs=1) as bigp,
            tc.tile_pool(name="xw", bufs=3) as xwp,
            tc.tile_pool(name="gt16", bufs=6) as gt16p,
            tc.tile_pool(name="sp", bufs=8) as sp,
            tc.tile_pool(name="tmp", bufs=4) as tmpp,
            tc.tile_pool(name="ap", bufs=4) as apool,
            tc.tile_pool(name="hp", bufs=4) as hpool,
            tc.tile_pool(name="pag", bufs=4, space="PSUM") as pag,
            tc.tile_pool(name="pz", bufs=2, space="PSUM") as pz,
            tc.tile_pool(name="pt", bufs=2, space="PSUM") as pt,
            tc.tile_pool(name="dram", bufs=1, space="DRAM") as dram,
        ):
            nc.gpsimd.load_library(library_config.mlp)

            # --- resident constants / inputs in SBUF ---
            idx_sb = []
            dof_sb = []
            for h in range(2):
                t = constp.tile([128, max(slots_h[h] // 16, 1)], i16, name=f"idxsb{h}")
                nc.sync.dma_start(out=t[:], in_=idx_d[h][:])
                idx_sb.append(t)
                t2 = constp.tile([128, max(nb_h[h], 1)], f32, name=f"dofsb{h}")
                nc.sync.dma_start(out=t2[:], in_=dof_d[h][:])
                dof_sb.append(t2)
            iota_sb = constp.tile([128, 8 * W_N], f16)
            nc.sync.dma_start(out=iota_sb[:], in_=iota_d[:])
            ident = constp.tile([128, 128], f32)
            make_identity(nc, ident[:])
            ident16 = constp.tile([128, 128], f16)
            nc.vector.tensor_copy(ident16[:], ident[:])
            embW_sb = constp.tile([128, KCH, HID], f16)
            nc.sync.dma_start(out=embW_sb[:], in_=embW_d[:].rearrange("k p h -> p k h"))
            embB_sb = constp.tile([128, 1], f32)
            nc.sync.dma_start(out=embB_sb[:], in_=embB_d[:])
            Wl_sb = constp.tile([128, L, HID], f16)
            nc.sync.dma_start(out=Wl_sb[:], in_=Wl_d[:].rearrange("l p h -> p l h"))
            Wr_sb = constp.tile([128, L, HID], f16)
            nc.sync.dma_start(out=Wr_sb[:], in_=Wr_d[:].rearrange("l p h -> p l h"))
            bl_sb = constp.tile([128, L], f32)
            nc.sync.dma_start(out=bl_sb[:], in_=bl_d[:].rearrange("l p one -> p (l one)"))
            inv_sb = bigp.tile([128, NPAD], f32)
            nc.sync.dma_start(out=inv_sb[:], in_=inv_d[:])

            hT = [bigp.tile([128, NPAD], f16, name=f"hT{i}") for i in range(2)]

            # DRAM buffers: per AG round, one tile per half-table chunk
            # (Shared-space DRAM requires a single writer per tile)
            n_ag = timing_reps * L
            cc_in = [
                dram.tile([NPAD, HID], f16, name=f"ccin{i}", bufs=1) for i in range(2)
            ]
            h_half = [
                [
                    dram.tile(
                        [HALF, HID], f16, name=f"hall{i}h{h}", bufs=1,
                        addr_space="Shared",
                    )
                    for h in range(2)
                ]
                for i in range(n_ag)
            ]

            def build_S_group(h, boff, nb, use_act):
                # one-hot S for nb consecutive blocks in one wide op:
                # S[p, b*W_N + c] = (iota[c] == dof[p, boff+b])
                S = sp.tile([128, nb * W_N], f16, tag="S", name="S")
                in0 = iota_sb[:, 0 : nb * W_N].rearrange(
                    "p (b c) -> p b c", c=W_N
                )
                d = dof_sb[h][:, boff : boff + nb]
                in1 = bass.AP(d.tensor, d.offset, list(d.ap) + [[0, W_N]])
                out3 = S[:].rearrange("p (b c) -> p b c", c=W_N)
                if use_act:
                    t = tmpp.tile([128, nb * W_N], f16, tag="T", name="T")
                    t3 = t[:].rearrange("p (b c) -> p b c", c=W_N)
                    # t = iota - dof ; S = relu(1 - |t|)  (exact one-hot)
                    nc.vector.tensor_tensor(
                        out=t3, in0=in0, in1=in1,
                        op=mybir.AluOpType.subtract,
                    )
                    nc.scalar.activation(
                        S[:], t[:], mybir.ActivationFunctionType.Abs,
                    )
                    nc.scalar.activation(
                        S[:], S[:], mybir.ActivationFunctionType.Relu,
                        bias=1.0, scale=-1.0,
                    )
                else:
                    nc.vector.tensor_tensor(
                        out=out3, in0=in0, in1=in1,
                        op=mybir.AluOpType.is_equal,
                    )
                return S

            def build_S(h, b_global, use_act):
                S = sp.tile([128, W_N], f16, tag="S", name="S")
                if use_act:
                    t = tmpp.tile([128, W_N], f16, tag="T", name="T")
                    # t = |dof - iota| ; S = relu(1 - t)  (exact one-hot)
                    nc.scalar.activation(
                        t[:], iota_sb[:, 0:W_N], mybir.ActivationFunctionType.Abs,
                        bias=dof_sb[h][:, b_global : b_global + 1], scale=-1.0,
                    )
                    nc.scalar.activation(
                        S[:], t[:], mybir.ActivationFunctionType.Relu,
                        bias=1.0, scale=-1.0,
                    )
                else:
                    nc.vector.tensor_scalar(
                        out=S[:],
                        in0=iota_sb[:, 0:W_N],
                        scalar1=dof_sb[h][:, b_global : b_global + 1],
                        scalar2=None,
                        op0=mybir.AluOpType.is_equal,
                    )
                return S

            def writeback_wide(hTbuf, w0, dest, last=False):
                cs = slice(w0 * W_N, (w0 + WB) * W_N)
                if last:
                    hsb = hpool.tile([128, WB * 128], f32, tag="hsbw32", name="hsbw32")
                else:
                    hsb = hpool.tile([128, WB * 128], f16, tag="hsbw", name="hsbw")
                for q in range(WB):
                    ptile = pt.tile([128, 128], f16, tag="pt16", name="ptile")
                    nc.tensor.transpose(
                        ptile[:],
                        hTbuf[:, (w0 + q) * W_N : (w0 + q + 1) * W_N],
                        ident16[:],
                    )
                    nc.any.tensor_copy(hsb[:, q * 128 : (q + 1) * 128], ptile[:])
                nc.sync.dma_start(
                    out=dest[cs, :].rearrange("(q p) n -> p q n", q=WB),
                    in_=hsb[:].rearrange("p (q n) -> p q n", q=WB),
                )

            def writeback(hTbuf, w, dest, last=False):
                # transpose window back to node-major and DMA to dest rows
                cs = slice(w * W_N, (w + 1) * W_N)
                ptile = pt.tile([128, 128], f16, tag="pt16", name="ptile")
                nc.tensor.transpose(ptile[:], hTbuf[:, cs], ident16[:])
                if last:
                    hsb = hpool.tile([128, 128], f32, tag="hsb32", name="hsb32")
                else:
                    hsb = hpool.tile([128, 128], f16, tag="hsb", name="hsb")
                nc.any.tensor_copy(hsb[:], ptile[:])
                nc.sync.dma_start(out=dest[cs, :], in_=hsb[:])

            def embedding(ag_pair):
                for w in range(NW):
                    ws = slice(w * W_N, (w + 1) * W_N)
                    xw = xwp.tile([128, KCH, W_N], f16, tag="xw", name="xw")
                    nc.sync.dma_start(
                        out=xw[:], in_=xT_d[:, :, ws].rearrange("k p n -> p k n")
                    )
                    pzz = pz.tile([128, W_N], f32, tag="pz", name="pz")
                    for k in range(KCH):
                        nc.tensor.matmul(
                            pzz[:],
                            lhsT=embW_sb[:, k, :],
                            rhs=xw[:, k, :],
                            start=(k == 0),
                            stop=(k == KCH - 1),
                        )
                    nc.scalar.activation(
                        hT[0][:, ws], pzz[:], mybir.ActivationFunctionType.Relu,
                        bias=embB_sb[:],
                    )
                    writeback(hT[0], w, cc_in[0])
                    if w in (NW // 2 - 1, NW - 1):
                        h = 0 if w == NW // 2 - 1 else 1
                        allgather_half(ag_pair[0], ag_pair[1], h)

            def layer(l, h_src, hT_in, hT_out, dest, last, ag_pair=None):
                half_ap = [h_src[0][:], h_src[1][:]]
                # issue gather chunks lazily as windows consume them
                gts = [[None] * len(chunks[0]), [None] * len(chunks[1])]

                def ensure_chunk(h, ci):
                    if gts[h][ci] is None:
                        w0, n = chunks[h][ci]
                        gt = gt16p.tile([128, n // 128, 128], f16, tag="gt", name="gt")
                        nc.gpsimd.dma_gather(
                            gt[:],
                            half_ap[h],
                            idx_sb[h][:, w0 // 16 : (w0 + n) // 16],
                            n,
                            n,
                            HID,
                            queue_num=next_q(),
                        )
                        gts[h][ci] = gt
                    return gts[h][ci]

                blk_ctr = 0
                for wg in range(NW // WB):
                    w0 = wg * WB
                    ws = slice(w0 * W_N, (w0 + WB) * W_N)
                    aggT = apool.tile([128, WB * W_N], f16, tag="aggT", name="aggT")
                    for wi in range(WB):
                        w = w0 + wi
                        pagg = pag.tile([128, W_N], f32, tag="pagg", name="pagg")
                        nblocks = int(B[0][w] + B[1][w])
                        done = 0
                        first = True
                        for h in range(2):
                            boff = int(B[h][:w].sum())
                            nb = int(B[h][w])
                            if _ablate:
                                for lb in range(nb):
                                    ci, j = blk_map[h][boff + lb]
                                    ensure_chunk(h, ci)
                                done += nb
                                continue
                            if nb == 0:
                                continue
                            use_act = (blk_ctr % ACT_MOD) >= ACT_CUT
                            blk_ctr += 1
                            Sg = build_S_group(h, boff, nb, use_act)
                            for lb in range(nb):
                                ci, j = blk_map[h][boff + lb]
                                gt = ensure_chunk(h, ci)
                                done += 1
                                nc.tensor.matmul(
                                    pagg[:],
                                    lhsT=gt[:, j, :],
                                    rhs=Sg[:, lb * W_N : (lb + 1) * W_N],
                                    start=first,
                                    stop=(done == nblocks),
                                )
                                first = False
                        asl = aggT[:, wi * W_N : (wi + 1) * W_N]
                        if first:
                            nc.vector.memset(asl, 0.0)
                        else:
                            nc.vector.tensor_tensor(
                                out=asl,
                                in0=pagg[:],
                                in1=inv_sb[:, (w0 + wi) * W_N : (w0 + wi + 1) * W_N],
                                op=mybir.AluOpType.mult,
                            )
                    pzz = pz.tile([128, WB * W_N], f32, tag="pz", name="pz")
                    nc.tensor.matmul(
                        pzz[:], lhsT=Wl_sb[:, l, :], rhs=aggT[:], start=True, stop=False
                    )
                    nc.tensor.matmul(
                        pzz[:], lhsT=Wr_sb[:, l, :], rhs=hT_in[:, ws], start=False,
                        stop=True,
                    )
                    nc.scalar.activation(
                        hT_out[:, ws], pzz[:], mybir.ActivationFunctionType.Relu,
                        bias=bl_sb[:, l : l + 1],
                    )
                    for q in range(WB):
                        writeback(hT_out, w0 + q, dest, last=last)
                    if ag_pair is not None and wg in (
                        (NW // 2 - 1) // WB, NW // WB - 1
                    ):
                        h = 0 if wg == (NW // 2 - 1) // WB else 1
                        allgather_half(ag_pair[0], ag_pair[1], h)

            def allgather_half(src_cc, dst_pair, h):
                if _no_ag:
                    return
                rs = slice(h * (NPAD // 2), (h + 1) * (NPAD // 2))
                nc.gpsimd.collective_compute(
                    "AllGather",
                    mybir.AluOpType.bypass,
                    replica_groups=rg,
                    ins=[src_cc[rs, :]],
                    outs=[dst_pair[h][:]],
                )

            embedding(ag_pair=(cc_in[0], h_half[0]))
            agi = 0
            nlay = timing_reps * L
            for li in range(nlay):
                l = li % L
                last = li == nlay - 1
                layer(
                    l,
                    h_half[agi],
                    hT[li % 2],
                    hT[(li + 1) % 2],
                    out_d if last else cc_in[(li + 1) % 2],
                    last,
                    ag_pair=None if last else (cc_in[(li + 1) % 2], h_half[agi + 1]),
                )
                if not last:
                    agi += 1

    nc.compile()
    return nc


def _prep_inputs(inputs, struct):
    x = np.asarray(inputs["x"], dtype=np.float32)
    emb_W = np.asarray(inputs["emb_W"], dtype=np.float32)
    emb_b = np.asarray(inputs["emb_b"], dtype=np.float32)
    Wl = np.asarray(inputs["Wl"], dtype=np.float32)
    bl = np.asarray(inputs["bl"], dtype=np.float32)
    Wr = np.asarray(inputs["Wr"], dtype=np.float32)

    embW_p = np.zeros((KCH, 128, HID), dtype=np.float16)
    embW_p.reshape(KCH * 128, HID)[:IN_DIM] = emb_W.astype(np.float16)
    embB_p = np.zeros((128, 1), dtype=np.float32)
    embB_p[:, 0] = emb_b
    Wl_p = Wl.astype(np.float16)
    Wr_p = Wr.astype(np.float16)
    bl_p = np.ascontiguousarray(bl[:, :, None].astype(np.float32))

    iota = np.broadcast_to(
        np.tile(np.arange(W_N, dtype=np.float16), 8)[None, :], (128, 8 * W_N)
    ).copy()

    inv_all = struct["inv_all"]

    in_maps = []
    for m in range(NC):
        xm = np.zeros((KCH * 128, NPAD), dtype=np.float16)
        xm[:IN_DIM, :NPC] = x[m * NPC : (m + 1) * NPC].T.astype(np.float16)
        inv_m = np.broadcast_to(
            inv_all[m * NPAD : (m + 1) * NPAD][None, :], (128, NPAD)
        ).copy()
        im = {
            "idx0": struct["idx_wrapped"][m][0],
            "idx1": struct["idx_wrapped"][m][1],
            "dof0": struct["dof"][m][0],
            "dof1": struct["dof"][m][1],
            "xT": xm.reshape(KCH, 128, NPAD),
            "embW": embW_p,
            "embB": embB_p,
            "Wl": Wl_p,
            "Wr": Wr_p,
            "bl": bl_p,
            "iota": iota,
            "inv": inv_m,
        }
        in_maps.append(im)
    return in_maps


class BassRunner:
    """Executes a compiled Bass program via PJRT/axon; jit built once."""

    def __init__(self, nc, n_cores):
        import jax
        from jax.sharding import Mesh, PartitionSpec
        from jax.experimental.shard_map import shard_map
        from concourse.bass2jax import (
            _bass_exec_p,
            install_neuronx_cc_hook,
            partition_id_tensor,
        )

        install_neuronx_cc_hook()
        self.jax = jax
        self.nc = nc
        self.n_cores = n_cores
        partition_name = (
            nc.partition_id_tensor.name if nc.partition_id_tensor else None
        )
        in_names, out_names, out_avals, zero_outs = [], [], [], []
        for alloc in nc.m.functions[0].allocations:
            if not isinstance(alloc, mybir.MemoryLocationSet):
                continue
            name = alloc.memorylocations[0].name
            if alloc.kind == "ExternalInput":
                if name != partition_name:
                    in_names.append(name)
            elif alloc.kind == "ExternalOutput":
                shape = tuple(alloc.tensor_shape)
                dtype = mybir.dt.np(alloc.dtype)
                out_names.append(name)
                out_avals.append(jax.core.ShapedArray(shape, dtype))
                zero_outs.append(np.zeros(shape, dtype))
        self.in_names, self.out_names = in_names, out_names
        self.zero_outs, self._out_avals = zero_outs, out_avals
        n_params, n_outs = len(in_names), len(out_avals)
        all_in_names = in_names + out_names
        if partition_name is not None:
            all_in_names = all_in_names + [partition_name]

        def _body(*args):
            operands = list(args)
            if partition_name is not None:
                operands.append(partition_id_tensor())
            return tuple(
                _bass_exec_p.bind(
                    *operands,
                    out_avals=tuple(out_avals),
                    in_names=tuple(all_in_names),
                    out_names=tuple(out_names),
                    lowering_input_output_aliases=(),
                    sim_require_finite=True,
                    sim_require_nnan=True,
                    nc=nc,
                )
            )

        devices = jax.devices()[:n_cores]
        self._mesh = Mesh(np.asarray(devices), ("core",))
        self._pspec = PartitionSpec("core")
        in_specs = (self._pspec,) * (n_params + n_outs)
        out_specs = (self._pspec,) * len(out_names)
        self._fn = jax.jit(
            shard_map(
                _body,
                mesh=self._mesh,
                in_specs=in_specs,
                out_specs=out_specs,
                check_rep=False,
            ),
            keep_unused=True,
        )

    def prepare(self, in_maps):
        n = self.n_cores
        concat_in = [
            np.concatenate(
                [np.asarray(in_maps[c][name]) for c in range(n)], axis=0
            )
            for name in self.in_names
        ]
        concat_zeros = [
            np.zeros((n * z.shape[0], *z.shape[1:]), z.dtype)
            for z in self.zero_outs
        ]
        sharding = self.jax.sharding.NamedSharding(self._mesh, self._pspec)
        self._args = [
            self.jax.device_put(a, sharding) for a in concat_in + concat_zeros
        ]

    def execute(self):
        outs = self._fn(*self._args)
        self.jax.block_until_ready(outs)
        return outs

    def run(self):
        outs = self.execute()
        n = self.n_cores
        return [
            {
                name: np.asarray(outs[i]).reshape(
                    n, *self._out_avals[i].shape
                )[c]
                for i, name in enumerate(self.out_names)
            }
            for c in range(n)
        ]


def _get_runner(edge_index, timing_reps=1):
    import os
    key = ("prog", timing_reps, os.environ.get("KERNEL_NO_AG"),
           os.environ.get("KERNEL_ABLATE"), hash(edge_index.tobytes()))
    if key in _CACHE:
        return _CACHE[key]
    struct = _host_prep(edge_index)
    nc = _build_program(struct, timing_reps=timing_reps)
    runner = BassRunner(nc, NC)
    _CACHE[key] = (struct, runner)
    return struct, runner


def kernel(**inputs):
    edge_index = np.asarray(inputs["edge_index"])
    struct, runner = _get_runner(edge_index)
    in_maps = _prep_inputs(inputs, struct)
    runner.prepare(in_maps)
    results = runner.run()
    out = np.empty((N, HID), dtype=np.float32)
    for m in range(NC):
        out[m * NPC : (m + 1) * NPC] = results[m]["out"][:NPC]
    return out


# revision 36
# speedup vs baseline: 1.4081x; 1.2892x over previous
"""GraphSAGE (mean aggregation) on 8 Trainium2 NeuronCores.

Strategy (v6):
  - Nodes partitioned across 8 cores (6250 real + pad -> 6400/core).
  - Full node-feature table [51200, 128] fp16 replicated in each core's
    DRAM as two 25600-row half-tables; refreshed with two fp16 AllGathers
    per layer. The half-0 AllGather fires mid-layer (after window 24's
    writeback) so it overlaps the remaining windows' compute; token ids
    are remapped (half = local//3200) so each AllGather chunk lands
    exactly in one gather half-table.
  - Edge messages fetched with dma_gather (SWDGE, 4 queues, 1024-index
    chunks spanning window boundaries, 2-deep ring via 32K scratch,
    int16 indices). fp16 rows feed the PE directly (no f32->f16 copy).
  - Mean aggregation = PE matmuls: per 128-edge block, lhsT = gathered
    messages [128e, 128h] fp16, rhs = one-hot S [128e, 128]; accumulated
    in a [128h, 128-node] PSUM window. S built in one wide op per
    (half, window) via a 0-stride broadcast AP (DVE is_equal; every 8th
    group on ACT via |iota-dof| -> relu(1-t), exact) -- batching S into
    ~100 ops/layer instead of ~700 is the single biggest HW win.
  - In-degree reciprocals precomputed on host, resident in SBUF; mean =
    psum * inv (DVE tensor_tensor).
  - Dense SAGE transform per window: zT = Wl^T aggT + Wr^T hT; bias+relu
    on ACT; h' transposed back to node-major via PE and DMAed to cc_in.
"""
import sys

sys.path.insert(0, "/opt/trn_rl_repo")

import numpy as np

import concourse.bass as bass
import concourse.bacc as bacc
import concourse.tile as tile
from concourse import mybir, library_config
from concourse.masks import make_identity

# problem constants (hardcoded per contract)
N, E, IN_DIM, HID, L = 50000, 625000, 300, 128, 4
NC = 8
NPC = N // NC            # 6250 real nodes per core
W_N = 128                # aggregation window width (psum free dim)
NW = 50                  # windows per core
NPAD = W_N * NW          # 6400 padded nodes per core
NTAB = NC * NPAD         # 51200 rows in the replicated table
HALF = NTAB // 2         # 25600 (int16 index limit per gather table slice)
HLOC = NPAD // 2         # 3200 locals per AG chunk/table half
KCH = 3                  # 384 = 3*128 >= IN_DIM contraction chunks
GMAX = 1024              # max indices per dma_gather (descriptor carveout)
SCRATCH = 32768          # dynamic dma scratch (ring = SCRATCH/16 descs/queue)
ACT_MOD = 8              # blocks i with i % ACT_MOD >= ACT_CUT go to ACT
ACT_CUT = 7              # 7/8 of S group builds on DVE, 1/8 on ACT
WB = 1                   # windows per dense/writeback batch


_CACHE = {}


def _host_prep(edge_index):
    """Build per-core gather streams, dst-offset blocks and program structure."""
    src = edge_index[0].astype(np.int64)
    dst = edge_index[1].astype(np.int64)
    # padded global ids
    gsrc = (src // NPC) * NPAD + (src % NPC)
    gdst = (dst // NPC) * NPAD + (dst % NPC)

    # token id within its half-table: half h = local//HLOC holds
    # rows m*HLOC + local%HLOC  (AG chunk h writes exactly half-table h)
    src_m = gsrc // NPAD
    src_loc = gsrc % NPAD
    src_half = src_loc // HLOC
    src_tok = src_m * HLOC + (src_loc % HLOC)

    per_core = []
    counts = np.zeros((NC, 2, NW), dtype=np.int64)
    for m in range(NC):
        sel = (gdst // NPAD) == m
        s_tok = src_tok[sel]
        dl = (gdst[sel] - m * NPAD).astype(np.int64)   # 0..6249
        half = src_half[sel]
        w = dl // W_N
        order = np.lexsort((dl, half))
        s_tok, dl, half, w = s_tok[order], dl[order], half[order], w[order]
        per_core.append((s_tok, dl, half, w))
        for h in range(2):
            cw = np.bincount(w[half == h], minlength=NW)
            counts[m, h, :] = cw

    # uniform block structure across cores
    B = np.zeros((2, NW), dtype=np.int64)
    for h in range(2):
        for w in range(NW):
            B[h, w] = int(np.ceil(counts[:, h, w].max() / 128.0))

    slots_h = [int(B[h].sum() * 128) for h in range(2)]
    nb_h = [int(B[h].sum()) for h in range(2)]

    # per-(h,w) slot offsets into the half stream
    slot_off = np.zeros((2, NW), dtype=np.int64)
    for h in range(2):
        acc = 0
        for w in range(NW):
            slot_off[h, w] = acc
            acc += B[h, w] * 128

    # gather chunks per half: runs of <= GMAX slots spanning window bounds
    chunks = [[], []]           # per half: list of (slot_off, n)
    for h in range(2):
        off, left = 0, slots_h[h]
        while left > 0:
            n = min(GMAX, left)
            chunks[h].append((off, n))
            off += n
            left -= n

    # map: (h, global_block) -> (chunk_id, j within chunk)
    blk_map = [[], []]
    for h in range(2):
        for b in range(nb_h[h]):
            s = b * 128
            for ci, (w0, n) in enumerate(chunks[h]):
                if w0 <= s < w0 + n:
                    blk_map[h].append((ci, (s - w0) // 128))
                    break

    # in-degrees (padded local layout) for the mean
    deg_all = np.bincount(gdst, minlength=NTAB).astype(np.float32)
    inv_all = 1.0 / np.maximum(deg_all, 1.0)

    idx_wrapped = []   # per core: [2][128, slots_h/16] int16
    dof_arr = []       # per core: [2][128, nb_h] fp32
    for m in range(NC):
        s_m, dl, half, w = per_core[m]
        iw_pair, dof_pair = [], []
        for h in range(2):
            tok = np.zeros(slots_h[h], dtype=np.int16)   # pad -> row 0
            dof = np.full(slots_h[h], -1.0, dtype=np.float32)
            sel = half == h
            s_h, dl_h, w_h = s_m[sel], dl[sel], w[sel]
            for wi in range(NW):
                selw = w_h == wi
                cnt = int(selw.sum())
                if cnt == 0:
                    continue
                o = int(slot_off[h, wi])
                tok[o : o + cnt] = s_h[selw].astype(np.int16)
                dof[o : o + cnt] = (dl_h[selw] - wi * W_N).astype(np.float32)
            # wrap idx per gather instruction: j -> [j%16, j//16], replicated x8
            iw = np.zeros((128, max(slots_h[h] // 16, 1)), dtype=np.int16)
            for w0, n in chunks[h]:
                blockw = tok[w0 : w0 + n].reshape(n // 16, 16).T  # [16, n/16]
                iw[:, w0 // 16 : (w0 + n) // 16] = np.tile(blockw, (8, 1))
            iw_pair.append(iw)
            # dstoff partition-major: dof_arr[p, b] = dof[b*128+p]
            dof_pair.append(
                np.ascontiguousarray(dof.reshape(max(nb_h[h], 1), 128).T)
            )
        idx_wrapped.append(iw_pair)
        dof_arr.append(dof_pair)

    return {
        "B": B,
        "slots_h": slots_h,
        "nb_h": nb_h,
        "slot_off": slot_off,
        "chunks": chunks,
        "blk_map": blk_map,
        "idx_wrapped": idx_wrapped,
        "dof": dof_arr,
        "inv_all": inv_all,
    }


def _build_program(struct, timing_reps=1):
    B = struct["B"]
    slots_h = struct["slots_h"]
    nb_h = struct["nb_h"]
    chunks = struct["chunks"]
    blk_map = struct["blk_map"]

    nc = bacc.Bacc(
        "TRN2",
        target_bir_lowering=False,
        debug=False,
        num_devices=NC,
        num_swdge_queues=4,
        dynamic_dma_scratch_size=SCRATCH,
    )
    f32, f16, i16 = mybir.dt.float32, mybir.dt.float16, mybir.dt.int16

    idx_d = [
        nc.dram_tensor(f"idx{h}", [128, max(slots_h[h] // 16, 1)], i16, kind="ExternalInput")
        for h in range(2)
    ]
    dof_d = [
        nc.dram_tensor(f"dof{h}", [128, max(nb_h[h], 1)], f32, kind="ExternalInput")
        for h in range(2)
    ]
    xT_d = nc.dram_tensor("xT", [KCH, 128, NPAD], f16, kind="ExternalInput")
    embW_d = nc.dram_tensor("embW", [KCH, 128, HID], f16, kind="ExternalInput")
    embB_d = nc.dram_tensor("embB", [128, 1], f32, kind="ExternalInput")
    Wl_d = nc.dram_tensor("Wl", [L, 128, HID], f16, kind="ExternalInput")
    Wr_d = nc.dram_tensor("Wr", [L, 128, HID], f16, kind="ExternalInput")
    bl_d = nc.dram_tensor("bl", [L, 128, 1], f32, kind="ExternalInput")
    iota_d = nc.dram_tensor("iota", [128, 8 * W_N], f16, kind="ExternalInput")
    inv_d = nc.dram_tensor("inv", [128, NPAD], f32, kind="ExternalInput")
    out_d = nc.dram_tensor("out", [NPAD, HID], f32, kind="ExternalOutput")

    rg = [list(range(NC))]
    qctr = [0]

    def next_q():
        q = qctr[0] % 4
        qctr[0] += 1
        return q

    import os as _os
    _trace = _os.environ.get("KERNEL_TRACE_SIM") == "1"
    _ablate = _os.environ.get("KERNEL_ABLATE") == "1"
    _no_ag = _os.environ.get("KERNEL_NO_AG") == "1"
    _sbatch = _os.environ.get("KERNEL_SBATCH", "1") == "1"
    _wbwide = _os.environ.get("KERNEL_WBWIDE", "1") == "1"
    with tile.TileContext(nc, trace_sim=_trace) as tc:
        with (
            tc.tile_pool(name="const", bufs=1) as constp,
            tc.tile_pool(name="big", bufs=1) as bigp,
            tc.tile_pool(name="xw", bufs=3) as xwp,
            tc.tile_pool(name="gt16", bufs=8) as gt16p,
            tc.tile_pool(name="sp", bufs=8) as sp,
            tc.tile_pool(name="tmp", bufs=4) as tmpp,
            tc.tile_pool(name="ap", bufs=4) as apool,
            tc.tile_pool(name="hp", bufs=4) as hpool,
            tc.tile_pool(name="pag", bufs=4, space="PSUM") as pag,
            tc.tile_pool(name="pz", bufs=2, space="PSUM") as pz,
            tc.tile_pool(name="pt", bufs=2, space="PSUM") as pt,
            tc.tile_pool(name="dram", bufs=1, space="DRAM") as dram,
        ):
            nc.gpsimd.load_library(library_config.mlp)

            # --- resident constants / inputs in SBUF ---
            idx_sb = []
            dof_sb = []
            for h in range(2):
                t = constp.tile([128, max(slots_h[h] // 16, 1)], i16, name=f"idxsb{h}")
                nc.sync.dma_start(out=t[:], in_=idx_d[h][:])
                idx_sb.append(t)
                t2 = constp.tile([128, max(nb_h[h], 1)], f32, name=f"dofsb{h}")
                nc.sync.dma_start(out=t2[:], in_=dof_d[h][:])
                dof_sb.append(t2)
            iota_sb = constp.tile([128, 8 * W_N], f16)
            nc.sync.dma_start(out=iota_sb[:], in_=iota_d[:])
            ident = constp.tile([128, 128], f32)
            make_identity(nc, ident[:])
            ident16 = constp.tile([128, 128], f16)
            nc.vector.tensor_copy(ident16[:], ident[:])
            embW_sb = constp.tile([128, KCH, HID], f16)
            nc.sync.dma_start(out=embW_sb[:], in_=embW_d[:].rearrange("k p h -> p k h"))
            embB_sb = constp.tile([128, 1], f32)
            nc.sync.dma_start(out=embB_sb[:], in_=embB_d[:])
            Wl_sb = constp.tile([128, L, HID], f16)
            nc.sync.dma_start(out=Wl_sb[:], in_=Wl_d[:].rearrange("l p h -> p l h"))
            Wr_sb = constp.tile([128, L, HID], f16)
            nc.sync.dma_start(out=Wr_sb[:], in_=Wr_d[:].rearrange("l p h -> p l h"))
            bl_sb = constp.tile([128, L], f32)
            nc.sync.dma_start(out=bl_sb[:], in_=bl_d[:].rearrange("l p one -> p (l one)"))
            inv_sb = bigp.tile([128, NPAD], f32)
            nc.sync.dma_start(out=inv_sb[:], in_=inv_d[:])

            hT = [bigp.tile([128, NPAD], f16, name=f"hT{i}") for i in range(2)]

            # DRAM buffers: per AG round, one tile per half-table chunk
            # (Shared-space DRAM requires a single writer per tile)
            n_ag = timing_reps * L
            cc_in = [
                dram.tile([NPAD, HID], f16, name=f"ccin{i}", bufs=1) for i in range(2)
            ]
            h_half = [
                [
                    dram.tile(
                        [HALF, HID], f16, name=f"hall{i}h{h}", bufs=1,
                        addr_space="Shared",
                    )
                    for h in range(2)
                ]
                for i in range(n_ag)
            ]

            def build_S_group(h, boff, nb, use_act):
                # one-hot S for nb consecutive blocks in one wide op:
                # S[p, b*W_N + c] = (iota[c] == dof[p, boff+b])
                S = sp.tile([128, nb * W_N], f16, tag="S", name="S")
                in0 = iota_sb[:, 0 : nb * W_N].rearrange(
                    "p (b c) -> p b c", c=W_N
                )
                d = dof_sb[h][:, boff : boff + nb]
                in1 = bass.AP(d.tensor, d.offset, list(d.ap) + [[0, W_N]])
                out3 = S[:].rearrange("p (b c) -> p b c", c=W_N)
                if use_act:
                    t = tmpp.tile([128, nb * W_N], f16, tag="T", name="T")
                    t3 = t[:].rearrange("p (b c) -> p b c", c=W_N)
                    # t = iota - dof ; S = relu(1 - |t|)  (exact one-hot)
                    nc.vector.tensor_tensor(
                        out=t3, in0=in0, in1=in1,
                        op=mybir.AluOpType.subtract,
                    )
                    nc.scalar.activation(
                        S[:], t[:], mybir.ActivationFunctionType.Abs,
                    )
                    nc.scalar.activation(
                        S[:], S[:], mybir.ActivationFunctionType.Relu,
                        bias=1.0, scale=-1.0,
                    )
                else:
                    nc.vector.tensor_tensor(
                        out=out3, in0=in0, in1=in1,
                        op=mybir.AluOpType.is_equal,
                    )
                return S

            def build_S(h, b_global, use_act):
                S = sp.tile([128, W_N], f16, tag="S", name="S")
                if use_act:
                    t = tmpp.tile([128, W_N], f16, tag="T", name="T")
                    # t = |dof - iota| ; S = relu(1 - t)  (exact one-hot)
                    nc.scalar.activation(
                        t[:], iota_sb[:, 0:W_N], mybir.ActivationFunctionType.Abs,
                        bias=dof_sb[h][:, b_global : b_global + 1], scale=-1.0,
                    )
                    nc.scalar.activation(
                        S[:], t[:], mybir.ActivationFunctionType.Relu,
                        bias=1.0, scale=-1.0,
                    )
                else:
                    nc.vector.tensor_scalar(
                        out=S[:],
                        in0=iota_sb[:, 0:W_N],
                        scalar1=dof_sb[h][:, b_global : b_global + 1],
                        scalar2=None,
                        op0=mybir.AluOpType.is_equal,
                    )
                return S

            def writeback_wide(hTbuf, w0, dest, last=False):
                cs = slice(w0 * W_N, (w0 + WB) * W_N)
                if last:
                    hsb = hpool.tile([128, WB * 128], f32, tag="hsbw32", name="hsbw32")
                else:
                    hsb = hpool.tile([128, WB * 128], f16, tag="hsbw", name="hsbw")
                for q in range(WB):
                    ptile = pt.tile([128, 128], f16, tag="pt16", name="ptile")
                    nc.tensor.transpose(
                        ptile[:],
                        hTbuf[:, (w0 + q) * W_N : (w0 + q + 1) * W_N],
                        ident16[:],
                    )
                    nc.any.tensor_copy(hsb[:, q * 128 : (q + 1) * 128], ptile[:])
                nc.sync.dma_start(
                    out=dest[cs, :].rearrange("(q p) n -> p q n", q=WB),
                    in_=hsb[:].rearrange("p (q n) -> p q n", q=WB),
                )

            def writeback(hTbuf, w, dest, last=False):
                # transpose window back to node-major and DMA to dest rows
                cs = slice(w * W_N, (w + 1) * W_N)
                ptile = pt.tile([128, 128], f16, tag="pt16", name="ptile")
                nc.tensor.transpose(ptile[:], hTbuf[:, cs], ident16[:])
                if last:
                    hsb = hpool.tile([128, 128], f32, tag="hsb32", name="hsb32")
                else:
                    hsb = hpool.tile([128, 128], f16, tag="hsb", name="hsb")
                nc.any.tensor_copy(hsb[:], ptile[:])
                nc.sync.dma_start(out=dest[cs, :], in_=hsb[:])

            def embedding(ag_pair):
                for w in range(NW):
                    ws = slice(w * W_N, (w + 1) * W_N)
                    xw = xwp.tile([128, KCH, W_N], f16, tag="xw", name="xw")
                    nc.sync.dma_start(
                        out=xw[:], in_=xT_d[:, :, ws].rearrange("k p n -> p k n")
                    )
                    pzz = pz.tile([128, W_N], f32, tag="pz", name="pz")
                    for k in range(KCH):
                        nc.tensor.matmul(
                            pzz[:],
                            lhsT=embW_sb[:, k, :],
                            rhs=xw[:, k, :],
                            start=(k == 0),
                            stop=(k == KCH - 1),
                        )
                    nc.scalar.activation(
                        hT[0][:, ws], pzz[:], mybir.ActivationFunctionType.Relu,
                        bias=embB_sb[:],
                    )
                    writeback(hT[0], w, cc_in[0])
                    if w in (NW // 2 - 1, NW - 1):
                        h = 0 if w == NW // 2 - 1 else 1
                        allgather_half(ag_pair[0], ag_pair[1], h)

            def layer(l, h_src, hT_in, hT_out, dest, last, ag_pair=None):
                half_ap = [h_src[0][:], h_src[1][:]]
                # issue gather chunks lazily as windows consume them
                gts = [[None] * len(chunks[0]), [None] * len(chunks[1])]

                def ensure_chunk(h, ci):
                    if gts[h][ci] is None:
                        w0, n = chunks[h][ci]
                        gt = gt16p.tile([128, n // 128, 128], f16, tag="gt", name="gt")
                        nc.gpsimd.dma_gather(
                            gt[:],
                            half_ap[h],
                            idx_sb[h][:, w0 // 16 : (w0 + n) // 16],
                            n,
                            n,
                            HID,
                            queue_num=next_q(),
                        )
                        gts[h][ci] = gt
                    return gts[h][ci]

                # half-0 chunk ids needed per window (for eager issue: the
                # half-1 gathers of the first windows wait on the previous
                # layer's tail AllGather at the in-order Pool sequencer, so
                # half-0 chunks must be requested ahead of them in program
                # order to keep the Pool busy during that wait)
                h0_last_chunk = [
                    blk_map[0][int(B[0][: w + 1].sum()) - 1][0]
                    if int(B[0][: w + 1].sum()) > 0
                    else -1
                    for w in range(NW)
                ]
                LA = 3  # windows of half-0 gather lookahead

                blk_ctr = 0
                for wg in range(NW // WB):
                    w0 = wg * WB
                    ws = slice(w0 * W_N, (w0 + WB) * W_N)
                    wla = min(w0 + LA, NW - 1)
                    for ci in range(h0_last_chunk[wla] + 1):
                        ensure_chunk(0, ci)
                    aggT = apool.tile([128, WB * W_N], f16, tag="aggT", name="aggT")
                    for wi in range(WB):
                        w = w0 + wi
                        pagg = pag.tile([128, W_N], f32, tag="pagg", name="pagg")
                        nblocks = int(B[0][w] + B[1][w])
                        done = 0
                        first = True
                        for h in range(2):
                            boff = int(B[h][:w].sum())
                            nb = int(B[h][w])
                            if _ablate:
                                for lb in range(nb):
                                    ci, j = blk_map[h][boff + lb]
                                    ensure_chunk(h, ci)
                                done += nb
                                continue
                            if nb == 0:
                                continue
                            use_act = (blk_ctr % ACT_MOD) >= ACT_CUT
                            blk_ctr += 1
                            Sg = build_S_group(h, boff, nb, use_act)
                            for lb in range(nb):
                                ci, j = blk_map[h][boff + lb]
                                gt = ensure_chunk(h, ci)
                                done += 1
                                nc.tensor.matmul(
                                    pagg[:],
                                    lhsT=gt[:, j, :],
                                    rhs=Sg[:, lb * W_N : (lb + 1) * W_N],
                                    start=first,
                                    stop=(done == nblocks),
                                )
                                first = False
                        asl = aggT[:, wi * W_N : (wi + 1) * W_N]
                        if first:
                            nc.vector.memset(asl, 0.0)
                        else:
                            nc.vector.tensor_tensor(
                                out=asl,
                                in0=pagg[:],
                                in1=inv_sb[:, (w0 + wi) * W_N : (w0 + wi + 1) * W_N],
                                op=mybir.AluOpType.mult,
                            )
                    pzz = pz.tile([128, WB * W_N], f32, tag="pz", name="pz")
                    nc.tensor.matmul(
                        pzz[:], lhsT=Wl_sb[:, l, :], rhs=aggT[:], start=True, stop=False
                    )
                    nc.tensor.matmul(
                        pzz[:], lhsT=Wr_sb[:, l, :], rhs=hT_in[:, ws], start=False,
                        stop=True,
                    )
                    nc.scalar.activation(
                        hT_out[:, ws], pzz[:], mybir.ActivationFunctionType.Relu,
                        bias=bl_sb[:, l : l + 1],
                    )
                    for q in range(WB):
                        writeback(hT_out, w0 + q, dest, last=last)
                    if ag_pair is not None and wg in (
                        (NW // 2 - 1) // WB, NW // WB - 1
                    ):
                        h = 0 if wg == (NW // 2 - 1) // WB else 1
                        allgather_half(ag_pair[0], ag_pair[1], h)

            def allgather_half(src_cc, dst_pair, h):
                if _no_ag:
                    return
                rs = slice(h * (NPAD // 2), (h + 1) * (NPAD // 2))
                nc.gpsimd.collective_compute(
                    "AllGather",
                    mybir.AluOpType.bypass,
                    replica_groups=rg,
                    ins=[src_cc[rs, :]],
                    outs=[dst_pair[h][:]],
                )

            embedding(ag_pair=(cc_in[0], h_half[0]))
            agi = 0
            nlay = timing_reps * L
            for li in range(nlay):
                l = li % L
                last = li == nlay - 1
                layer(
                    l,
                    h_half[agi],
                    hT[li % 2],
                    hT[(li + 1) % 2],
                    out_d if last else cc_in[(li + 1) % 2],
                    last,
                    ag_pair=None if last else (cc_in[(li + 1) % 2], h_half[agi + 1]),
                )
                if not last:
                    agi += 1

    nc.compile()
    return nc


def _prep_inputs(inputs, struct):
    x = np.asarray(inputs["x"], dtype=np.float32)
    emb_W = np.asarray(inputs["emb_W"], dtype=np.float32)
    emb_b = np.asarray(inputs["emb_b"], dtype=np.float32)
    Wl = np.asarray(inputs["Wl"], dtype=np.float32)
    bl = np.asarray(inputs["bl"], dtype=np.float32)
    Wr = np.asarray(inputs["Wr"], dtype=np.float32)

    embW_p = np.zeros((KCH, 128, HID), dtype=np.float16)
    embW_p.reshape(KCH * 128, HID)[:IN_DIM] = emb_W.astype(np.float16)
    embB_p = np.zeros((128, 1), dtype=np.float32)
    embB_p[:, 0] = emb_b
    Wl_p = Wl.astype(np.float16)
    Wr_p = Wr.astype(np.float16)
    bl_p = np.ascontiguousarray(bl[:, :, None].astype(np.float32))

    iota = np.broadcast_to(
        np.tile(np.arange(W_N, dtype=np.float16), 8)[None, :], (128, 8 * W_N)
    ).copy()

    inv_all = struct["inv_all"]

    in_maps = []
    for m in range(NC):
        xm = np.zeros((KCH * 128, NPAD), dtype=np.float16)
        xm[:IN_DIM, :NPC] = x[m * NPC : (m + 1) * NPC].T.astype(np.float16)
        inv_m = np.broadcast_to(
            inv_all[m * NPAD : (m + 1) * NPAD][None, :], (128, NPAD)
        ).copy()
        im = {
            "idx0": struct["idx_wrapped"][m][0],
            "idx1": struct["idx_wrapped"][m][1],
            "dof0": struct["dof"][m][0],
            "dof1": struct["dof"][m][1],
            "xT": xm.reshape(KCH, 128, NPAD),
            "embW": embW_p,
            "embB": embB_p,
            "Wl": Wl_p,
            "Wr": Wr_p,
            "bl": bl_p,
            "iota": iota,
            "inv": inv_m,
        }
        in_maps.append(im)
    return in_maps


class BassRunner:
    """Executes a compiled Bass program via PJRT/axon; jit built once."""

    def __init__(self, nc, n_cores):
        import jax
        from jax.sharding import Mesh, PartitionSpec
        from jax.experimental.shard_map import shard_map
        from concourse.bass2jax import (
            _bass_exec_p,
            install_neuronx_cc_hook,
            partition_id_tensor,
        )

        install_neuronx_cc_hook()
        self.jax = jax
        self.nc = nc
        self.n_cores = n_cores
        partition_name = (
            nc.partition_id_tensor.name if nc.partition_id_tensor else None
        )
        in_names, out_names, out_avals, zero_outs = [], [], [], []
        for alloc in nc.m.functions[0].allocations:
            if not isinstance(alloc, mybir.MemoryLocationSet):
                continue
            name = alloc.memorylocations[0].name
            if alloc.kind == "ExternalInput":
                if name != partition_name:
                    in_names.append(name)
            elif alloc.kind == "ExternalOutput":
                shape = tuple(alloc.tensor_shape)
                dtype = mybir.dt.np(alloc.dtype)
                out_names.append(name)
                out_avals.append(jax.core.ShapedArray(shape, dtype))
                zero_outs.append(np.zeros(shape, dtype))
        self.in_names, self.out_names = in_names, out_names
        self.zero_outs, self._out_avals = zero_outs, out_avals
        n_params, n_outs = len(in_names), len(out_avals)
        all_in_names = in_names + out_names
        if partition_name is not None:
            all_in_names = all_in_names + [partition_name]

        def _body(*args):
            operands = list(args)
            if partition_name is not None:
                operands.append(partition_id_tensor())
            return tuple(
                _bass_exec_p.bind(
                    *operands,
                    out_avals=tuple(out_avals),
                    in_names=tuple(all_in_names),
                    out_names=tuple(out_names),
                    lowering_input_output_aliases=(),
                    sim_require_finite=True,
                    sim_require_nnan=True,
                    nc=nc,
                )
            )

        devices = jax.devices()[:n_cores]
        self._mesh = Mesh(np.asarray(devices), ("core",))
        self._pspec = PartitionSpec("core")
        in_specs = (self._pspec,) * (n_params + n_outs)
        out_specs = (self._pspec,) * len(out_names)
        self._fn = jax.jit(
            shard_map(
                _body,
                mesh=self._mesh,
                in_specs=in_specs,
                out_specs=out_specs,
                check_rep=False,
            ),
            keep_unused=True,
        )

    def prepare(self, in_maps):
        n = self.n_cores
        concat_in = [
            np.concatenate(
                [np.asarray(in_maps[c][name]) for c in range(n)], axis=0
            )
            for name in self.in_names
        ]
        concat_zeros = [
            np.zeros((n * z.shape[0], *z.shape[1:]), z.dtype)
            for z in self.zero_outs
        ]
        sharding = self.jax.sharding.NamedSharding(self._mesh, self._pspec)
        self._args = [
            self.jax.device_put(a, sharding) for a in concat_in + concat_zeros
        ]

    def execute(self):
        outs = self._fn(*self._args)
        self.jax.block_until_ready(outs)
        return outs

    def run(self):
        outs = self.execute()
        n = self.n_cores
        return [
            {
                name: np.asarray(outs[i]).reshape(
                    n, *self._out_avals[i].shape
                )[c]
                for i, name in enumerate(self.out_names)
            }
            for c in range(n)
        ]


def _get_runner(edge_index, timing_reps=1):
    import os
    key = ("prog", timing_reps, os.environ.get("KERNEL_NO_AG"),
           os.environ.get("KERNEL_ABLATE"), hash(edge_index.tobytes()))
    if key in _CACHE:
        return _CACHE[key]
    struct = _host_prep(edge_index)
    nc = _build_program(struct, timing_reps=timing_reps)
    runner = BassRunner(nc, NC)
    _CACHE[key] = (struct, runner)
    return struct, runner


def kernel(**inputs):
    edge_index = np.asarray(inputs["edge_index"])
    struct, runner = _get_runner(edge_index)
    in_maps = _prep_inputs(inputs, struct)
    runner.prepare(in_maps)
    results = runner.run()
    out = np.empty((N, HID), dtype=np.float32)
    for m in range(NC):
        out[m * NPC : (m + 1) * NPC] = results[m]["out"][:NPC]
    return out
